# revision 3
# baseline (speedup 1.0000x reference)
"""Sauvola binarization kernel v2 for 8 Trainium2 NeuronCores (data-parallel).

Structure (per core, one 1024x1024x3 image):
  pass 1 over 16 half-stripes (w-stripe j, row-half h): DMA [128,4,384],
    u1/u2 grayscale (DVE f32, some u1 on POOL), gray16 cast (DVE),
    g2c=(51g-25.5)^2 (ACT), exact f32 min/max folds (POOL),
    banded H-boxsum matmuls (PE) -> merged ta|tb fp16 copy (ACT).
  global r: DVE final reduces -> partition_all_reduce (POOL) -> AllGather
    (15us, vs 28us AllReduce) -> broadcast-read -> slot max-reduce.
    1/(2r) is folded into the mask compare: mask = (v1 * rsum2) > CV2*s0*qa.
  pass 2 in (m-block, col-half) units of [128,512] PSUM: W-boxsum matmuls
    (PE), t1/sqrt (ACT), t2/v1/v2 (DVE stt), masks split DVE/POOL.
  Left-half units only need stripes 0..4, so they overlap pass 1.
"""
import numpy as np

import concourse.bass as bass
import concourse.mybir as mybir
import concourse.tile as tile
from concourse.bass_utils import run_bass_kernel_spmd

N_CORES = 8
F = mybir.dt.float32
Hh = mybir.dt.float16
W0, W1, W2 = 0.2989, 0.5870, 0.1140
KS = 0.2
HALF = 25
WINDOWS = [(0, 0, 153), (1, 103, 178), (2, 231, 178), (3, 359, 153), (3, 512, 25),
           (4, 487, 25), (4, 512, 153), (5, 615, 178), (6, 743, 178), (7, 871, 153)]
WOFF = [0]
for _i, _c, _n in WINDOWS[:-1]:
    WOFF.append(WOFF[-1] + _n)
BAND_W = WOFF[-1] + WINDOWS[-1][2]   # 1374
# pass-2 col halves: pieces whose output range is in [0,512) vs [512,1024)
H0_KS = [0, 1, 2, 3, 5]
H1_KS = [4, 6, 7, 8, 9]
P0PP = (1.0 - KS) / (2601.0 * W0)
# mask:  v1 * ((maxu2-minu2)*W0/K2) > CV2*s0*qa,  s0 = 2601*s*2^-7
CV2 = 2.0 ** -4
K2 = 2.0 * KS * (2 ** 7) / (W0 * 2601.0 ** 2 * CV2)
RS_SCALE = 1.0 / K2   # pair is in gray units
U1_POOL_STRIPES = ()                 # u1 stays on DVE
MASK_POOL_UNITS = 6                  # of 16 mask units, this many on POOL


def _split_multi_waits(nc):
    """walrus here allows one sync wait per instruction; split extras to NOPs."""
    for func in nc.m.functions:
        for bb in func.blocks:
            insts = bb.instructions
            i = 0
            while i < len(insts):
                inst = insts[i]
                si = inst.sync_info
                if si is None or len(si.on_wait) <= 1:
                    i += 1
                    continue
                waits = list(si.on_wait)
                nops = []
                for w in waits[:-1]:
                    nop = mybir.InstNoOp(
                        name=nc.get_next_instruction_name(),
                        sync_info=mybir.SyncInfo(on_wait=[w], on_update=[]),
                        bass_nofuse=True,
                        engine=inst.engine,
                    )
                    nops.append(nop)
                inst.sync_info = mybir.SyncInfo(
                    on_wait=[waits[-1]], on_update=list(si.on_update)
                )
                for k, nop in enumerate(nops):
                    insts.insert(i + k, nop)
                    nc.register_instruction(nop, overwrite=True)
                i += len(nops) + 1


def _build_band_blocks():
    B = np.zeros((1024, 1024), dtype=np.float32)
    idx = np.arange(1024)
    for d in range(-HALF, HALF + 1):
        t = idx + d
        t = np.where(t < 0, -t, t)
        t = np.where(t > 1023, 2046 - t, t)
        np.add.at(B, (idx, t), 1.0)
    blocks = np.zeros((128, BAND_W), dtype=np.float16)
    for k, (i, c0, ncols) in enumerate(WINDOWS):
        blocks[:, WOFF[k]:WOFF[k] + ncols] = B[c0:c0 + ncols,
                                               128 * i:128 * (i + 1)].T[:, :]
    return blocks


def _build_nc():
    nc = bass.Bass("TRN2", target_bir_lowering=False, debug=False,
                   num_devices=N_CORES)
    x = nc.dram_tensor("x", [1024, 3072], F, kind="ExternalInput")
    band = nc.dram_tensor("band", [128, BAND_W], Hh, kind="ExternalInput")
    out = nc.dram_tensor("out", [1024, 1024], Hh, kind="ExternalOutput")

    AluOp = mybir.AluOpType
    Act = mybir.ActivationFunctionType

    with tile.TileContext(nc) as tc:
        with (
            tc.tile_pool(name="consts", bufs=1) as consts,
            tc.tile_pool(name="xin", bufs=4) as xin,
            tc.tile_pool(name="work", bufs=6) as work,
            tc.tile_pool(name="qpool", bufs=6) as qpool,
            tc.tile_pool(name="keep", bufs=1) as keep,
            tc.tile_pool(name="grayp", bufs=3) as grayp,
            tc.tile_pool(name="tkeep", bufs=8) as tkeep,
            tc.tile_pool(name="vkeep", bufs=1) as vkeep,
            tc.tile_pool(name="maskp", bufs=4) as maskp,
            tc.tile_pool(name="ps1", bufs=1, space="PSUM") as ps1,
            tc.tile_pool(name="ps2", bufs=2, space="PSUM") as ps2,
            tc.tile_pool(name="dram", bufs=1, space="DRAM") as dram,
        ):
            xc = x.ap().rearrange("(i p) (j w) -> p i j w", p=128, w=384)
            # first half-stripe DMA goes before the band const DMA
            xtiles = {}
            xt00 = xin.tile([128, 4, 384], F, tag="xj")
            nc.sync.dma_start(xt00[:], xc[:, 0:4, 0, :])
            xtiles[(0, 0)] = xt00

            band_sb = consts.tile([128, BAND_W], Hh)
            nc.sync.dma_start(band_sb[:], band.ap())
            bias_sq = consts.tile([128, 1], F)
            nc.gpsimd.memset(bias_sq[:], -25.5)
            bias_t1 = consts.tile([128, 1], F)
            nc.gpsimd.memset(bias_t1[:], -1300.5 * 2.0 ** -7)
            gmaxs = consts.tile([1, 9], F)
            gmins = consts.tile([1, 9], F)

            u2all = keep.tile([128, 8, 8, 128], F)       # gray / W0, all pixels
            ta_tiles, tb_tiles = [], []
            v1m, v2m = {}, {}
            for m in range(8):
                t_v1 = vkeep.tile([128, 1024], Hh, name=f"v1_{m}")
                t_v2 = vkeep.tile([128, 1024], Hh, name=f"v2_{m}")
                v1m[m], v2m[m] = t_v1, t_v2

            def emit_p2_unit(m, hc):
                """pass-2 unit: rows block m, cols [512*hc, 512*hc+512)."""
                base = 512 * hc
                ks = H0_KS if hc == 0 else H1_KS
                qa = ps2.tile([128, 512], F, tag="qa")
                qb = ps2.tile([128, 512], F, tag="qb")
                for src_tiles, pt in ((ta_tiles, qa), (tb_tiles, qb)):
                    for n, k in enumerate(ks):
                        jj, c0, ncols = WINDOWS[k]
                        nc.tensor.matmul(
                            pt[:, c0 - base:c0 - base + ncols],
                            src_tiles[jj][:, 128 * m:128 * (m + 1)],
                            band_sb[:, WOFF[k]:WOFF[k] + ncols],
                            start=(n == 0), stop=(n == len(ks) - 1))
                # t1 = ((qa - 1300.5) * 2^-7)^2   (fp16)
                t1 = work.tile([128, 512], Hh, tag="t1")
                nc.scalar.activation(t1[:], qa[:], Act.Square,
                                     bias=bias_t1[:], scale=2.0 ** -7)
                # t2 = qb*2^-14 - t1  = 2601^2 var * 2^-14  (fp16)
                t2 = work.tile([128, 512], Hh, tag="t2")
                nc.vector.scalar_tensor_tensor(
                    t2[:], qb[:], 2.0 ** -14, t1[:],
                    op0=AluOp.mult, op1=AluOp.subtract)
                # s0 = sqrt(t2) = 2601 * s * 2^-7  (fp16)
                s0 = work.tile([128, 512], Hh, tag="s0")
                nc.scalar.activation(s0[:], t2[:], Act.Sqrt, scale=1.0)
                v1 = v1m[m][:, 512 * hc:512 * hc + 512]
                u2slice = u2all[:, m, 4 * hc:4 * hc + 4, :].rearrange(
                    "p a b -> p (a b)")
                nc.vector.scalar_tensor_tensor(
                    v1, qa[:], -P0PP, u2slice,
                    op0=AluOp.mult, op1=AluOp.subtract)
                # v2C = CV2 * s0 * qa  (fp16)
                v2 = v2m[m][:, 512 * hc:512 * hc + 512]
                nc.vector.scalar_tensor_tensor(
                    v2, s0[:], CV2, qa[:],
                    op0=AluOp.mult, op1=AluOp.mult)

            # ---------------- pass 1: 16 half-stripes ----------------
            for j in range(8):
                tab = ps1.tile([128, 2048], F, tag="AB")
                xa = xtiles.get((j, 0))
                if xa is None:
                    xa = xin.tile([128, 4, 384], F, tag="xj")
                    nc.sync.dma_start(xa[:], xc[:, 0:4, j, :])
                xb = xin.tile([128, 4, 384], F, tag="xj")
                nc.sync.dma_start(xb[:], xc[:, 4:8, j, :])
                s3a = xa[:].rearrange("p i (w c) -> p i w c", c=3)
                s3b = xb[:].rearrange("p i (w c) -> p i w c", c=3)

                u1 = work.tile([128, 8, 128], F, tag="u1")
                u2 = u2all[:, :, j, :]
                gray = grayp.tile([128, 8, 128], Hh, tag="gray")
                g2c = grayp.tile([128, 8, 128], Hh, tag="g2c")
                halves = ((0, s3a), (1, s3b)) if j == 7 else ((2, None),)
                for hh, s3h in halves:
                    if hh == 2:   # full-stripe ops
                        sl = slice(0, 8)
                        src1, src0 = s3a, s3b
                        # u1n = -u1, u2n = -u2 (negated chain, same cost)
                        nc.vector.scalar_tensor_tensor(
                            u1[:, 0:4], s3a[:, :, :, 1], -W1 / W0,
                            s3a[:, :, :, 0], op0=AluOp.mult,
                            op1=AluOp.subtract)
                        nc.vector.scalar_tensor_tensor(
                            u1[:, 4:8], s3b[:, :, :, 1], -W1 / W0,
                            s3b[:, :, :, 0], op0=AluOp.mult,
                            op1=AluOp.subtract)
                        nc.vector.scalar_tensor_tensor(
                            u2[:, 0:4], s3a[:, :, :, 2], -W2 / W0, u1[:, 0:4],
                            op0=AluOp.mult, op1=AluOp.add)
                        nc.vector.scalar_tensor_tensor(
                            u2[:, 4:8], s3b[:, :, :, 2], -W2 / W0, u1[:, 4:8],
                            op0=AluOp.mult, op1=AluOp.add)
                        nc.vector.tensor_scalar(gray[:], u2, -W0, None,
                                                op0=AluOp.mult)
                        nc.scalar.activation(g2c[:], gray[:], Act.Square,
                                             bias=bias_sq[:], scale=51.0)
                        # max(gray) from fp16 gray; min via max of -u2 (exact)
                        nc.gpsimd.tensor_reduce(
                            gmaxs[0:1, j:j + 1], gray[:],
                            mybir.AxisListType.XYZWC, AluOp.max)
                        nc.gpsimd.tensor_reduce(
                            gmins[0:1, j:j + 1], u2,
                            mybir.AxisListType.XYZWC, AluOp.max)
                    else:         # stripe 7: half-granularity for a short tail
                        sl = slice(4 * hh, 4 * hh + 4)
                        nc.vector.scalar_tensor_tensor(
                            u1[:, sl], s3h[:, :, :, 1], -W1 / W0,
                            s3h[:, :, :, 0], op0=AluOp.mult,
                            op1=AluOp.subtract)
                        nc.vector.scalar_tensor_tensor(
                            u2[:, sl], s3h[:, :, :, 2], -W2 / W0, u1[:, sl],
                            op0=AluOp.mult, op1=AluOp.add)
                        nc.vector.tensor_scalar(gray[:, sl], u2[:, sl], -W0,
                                                None, op0=AluOp.mult)
                        nc.scalar.activation(g2c[:, sl], gray[:, sl],
                                             Act.Square, bias=bias_sq[:],
                                             scale=51.0)
                        with tc.high_priority():
                            nc.gpsimd.tensor_reduce(
                                gmaxs[0:1, 8:9] if hh else gmaxs[0:1, 7:8],
                                gray[:, sl], mybir.AxisListType.XYZWC,
                                AluOp.max)
                            nc.gpsimd.tensor_reduce(
                                gmins[0:1, 8:9] if hh else gmins[0:1, 7:8],
                                u2[:, sl], mybir.AxisListType.XYZWC,
                                AluOp.max)

                # H-pass banded matmuls
                for src, off in ((gray, 0), (g2c, 1024)):
                    for k, (i, c0, ncols) in enumerate(WINDOWS):
                        nc.tensor.matmul(
                            tab[:, off + c0:off + c0 + ncols],
                            src[:, i, :],
                            band_sb[:, WOFF[k]:WOFF[k] + ncols],
                            start=(k in (0, 4)), stop=(k in (5, 9)))
                # merged ta|tb fp16 copy
                tabh = tkeep.tile([128, 2048], Hh, tag="tab")
                nc.scalar.copy(tabh[:], tab[:])
                ta_tiles.append(tabh[:, 0:1024])
                tb_tiles.append(tabh[:, 1024:2048])


            # ------------- global r (AllGather of (gmax, -gmin)) ------
            # entire final chain on POOL: its queue is empty here, while
            # DVE's exec queue is flooded with pass-2 unit work
            with tc.high_priority():
                # gmaxs slots are gray units; gmins slots hold max(-u2)=-gmin
                gmaxu = consts.tile([1, 1], F)
                nc.gpsimd.tensor_reduce(gmaxu[:], gmaxs[:],
                                        mybir.AxisListType.XYZWC, AluOp.max)
                gminu = consts.tile([1, 1], F)
                nc.gpsimd.tensor_reduce(gminu[:], gmins[:],
                                        mybir.AxisListType.XYZWC, AluOp.max)
                pairg = consts.tile([1, 2], F)
                nc.gpsimd.tensor_scalar(pairg[:, 0:1], gmaxu[:], 1.0, None,
                                        op0=AluOp.mult)
                nc.gpsimd.tensor_scalar(pairg[:, 1:2], gminu[:], W0, None,
                                        op0=AluOp.mult)
                mm_in = dram.tile([1, 2], F)
                mm_sh = dram.tile([1, 16], F, addr_space="Shared")
                nc.sync.dma_start(mm_in[:], pairg[:])
                nc.gpsimd.collective_compute(
                    "AllGather", AluOp.bypass,
                    replica_groups=[list(range(N_CORES))],
                    ins=[mm_in.opt()], outs=[mm_sh.opt()])

            # ---- remaining pass-2 units (before the r-dependent chain,
            # so DVE/ACT streams do not block on the collective) ----
            for m in range(0, 8):
                emit_p2_unit(m, 0)
            for m in range(8):
                emit_p2_unit(m, 1)

            # ---- r-dependent chain ----
            with tc.high_priority():
                mm_b = consts.tile([128, 16], F)
                nc.sync.dma_start(mm_b[:], mm_sh[:].to_broadcast((128, 16)))
                glob = consts.tile([128, 2], F)
                nc.vector.tensor_reduce(
                    glob[:], mm_b[:].rearrange("p (s c) -> p c s", c=2),
                    mybir.AxisListType.X, AluOp.max)
                rs1 = consts.tile([128, 1], F)
                nc.vector.tensor_tensor(rs1[:], glob[:, 0:1], glob[:, 1:2],
                                        op=AluOp.add)
                rsum2 = consts.tile([128, 1], F)
                nc.vector.tensor_scalar(rsum2[:], rs1[:], RS_SCALE, None,
                                        op0=AluOp.mult)

            # masks: w = v1*rsum2 (ACT, per-partition scale), then
            # mask = w > v2C (DVE fp16 TT 2x)
            out_r = out.ap().rearrange("(m p) w -> m p w", p=128)
            for m in range(8):
                w = maskp.tile([128, 1024], Hh, tag="w")
                nc.scalar.activation(w[:], v1m[m][:], Act.Copy,
                                     scale=rsum2[:])
                mask = maskp.tile([128, 1024], Hh, tag="mask")
                nc.vector.tensor_tensor(mask[:], w[:], v2m[m][:],
                                        op=AluOp.is_gt)
                nc.sync.dma_start(out_r[m], mask[:])

    _split_multi_waits(nc)
    return nc


_CACHE = {}


def _get_nc():
    if "nc" not in _CACHE:
        _CACHE["nc"] = _build_nc()
        _CACHE["band"] = _build_band_blocks()
    return _CACHE["nc"], _CACHE["band"]


def kernel(inputs: np.ndarray) -> np.ndarray:
    nc, band = _get_nc()
    x = np.asarray(inputs, dtype=np.float32)
    in_maps = [
        {"x": np.ascontiguousarray(x[c].reshape(1024, 3072)), "band": band}
        for c in range(N_CORES)
    ]
    res = run_bass_kernel_spmd(nc, in_maps, list(range(N_CORES)))
    masks = [res.results[c]["out"] for c in range(N_CORES)]
    return np.stack(masks)[..., None].astype(np.float32)


# revision 4
# speedup vs baseline: 1.0012x; 1.0012x over previous
"""Sauvola binarization kernel v2 for 8 Trainium2 NeuronCores (data-parallel).

Structure (per core, one 1024x1024x3 image):
  pass 1 over 16 half-stripes (w-stripe j, row-half h): DMA [128,4,384],
    u1/u2 grayscale (DVE f32, some u1 on POOL), gray16 cast (DVE),
    g2c=(51g-25.5)^2 (ACT), exact f32 min/max folds (POOL),
    banded H-boxsum matmuls (PE) -> merged ta|tb fp16 copy (ACT).
  global r: DVE final reduces -> partition_all_reduce (POOL) -> AllGather
    (15us, vs 28us AllReduce) -> broadcast-read -> slot max-reduce.
    1/(2r) is folded into the mask compare: mask = (v1 * rsum2) > CV2*s0*qa.
  pass 2 in (m-block, col-half) units of [128,512] PSUM: W-boxsum matmuls
    (PE), t1/sqrt (ACT), t2/v1/v2 (DVE stt), masks split DVE/POOL.
  Left-half units only need stripes 0..4, so they overlap pass 1.
"""
import numpy as np

import concourse.bass as bass
import concourse.mybir as mybir
import concourse.tile as tile
from concourse.bass_utils import run_bass_kernel_spmd

N_CORES = 8
F = mybir.dt.float32
Hh = mybir.dt.float16
W0, W1, W2 = 0.2989, 0.5870, 0.1140
KS = 0.2
HALF = 25
WINDOWS = [(0, 0, 153), (1, 103, 178), (2, 231, 178), (3, 359, 153), (3, 512, 25),
           (4, 487, 25), (4, 512, 153), (5, 615, 178), (6, 743, 178), (7, 871, 153)]
WOFF = [0]
for _i, _c, _n in WINDOWS[:-1]:
    WOFF.append(WOFF[-1] + _n)
BAND_W = WOFF[-1] + WINDOWS[-1][2]   # 1374
# pass-2 col halves: pieces whose output range is in [0,512) vs [512,1024)
H0_KS = [0, 1, 2, 3, 5]
H1_KS = [4, 6, 7, 8, 9]
P0PP = (1.0 - KS) / (2601.0 * W0)
# mask:  v1 * ((maxu2-minu2)*W0/K2) > CV2*s0*qa,  s0 = 2601*s*2^-7
CV2 = 2.0 ** -4
K2 = 2.0 * KS * (2 ** 7) / (W0 * 2601.0 ** 2 * CV2)
RS_SCALE = 1.0 / K2   # pair is in gray units
U1_POOL_STRIPES = ()                 # u1 stays on DVE
MASK_POOL_UNITS = 6                  # of 16 mask units, this many on POOL


def _split_multi_waits(nc):
    """walrus here allows one sync wait per instruction; split extras to NOPs."""
    for func in nc.m.functions:
        for bb in func.blocks:
            insts = bb.instructions
            i = 0
            while i < len(insts):
                inst = insts[i]
                si = inst.sync_info
                if si is None or len(si.on_wait) <= 1:
                    i += 1
                    continue
                waits = list(si.on_wait)
                nops = []
                for w in waits[:-1]:
                    nop = mybir.InstNoOp(
                        name=nc.get_next_instruction_name(),
                        sync_info=mybir.SyncInfo(on_wait=[w], on_update=[]),
                        bass_nofuse=True,
                        engine=inst.engine,
                    )
                    nops.append(nop)
                inst.sync_info = mybir.SyncInfo(
                    on_wait=[waits[-1]], on_update=list(si.on_update)
                )
                for k, nop in enumerate(nops):
                    insts.insert(i + k, nop)
                    nc.register_instruction(nop, overwrite=True)
                i += len(nops) + 1


def _build_band_blocks():
    B = np.zeros((1024, 1024), dtype=np.float32)
    idx = np.arange(1024)
    for d in range(-HALF, HALF + 1):
        t = idx + d
        t = np.where(t < 0, -t, t)
        t = np.where(t > 1023, 2046 - t, t)
        np.add.at(B, (idx, t), 1.0)
    blocks = np.zeros((128, BAND_W), dtype=np.float16)
    for k, (i, c0, ncols) in enumerate(WINDOWS):
        blocks[:, WOFF[k]:WOFF[k] + ncols] = B[c0:c0 + ncols,
                                               128 * i:128 * (i + 1)].T[:, :]
    return blocks


def _build_nc():
    nc = bass.Bass("TRN2", target_bir_lowering=False, debug=False,
                   num_devices=N_CORES)
    x = nc.dram_tensor("x", [1024, 3072], F, kind="ExternalInput")
    band = nc.dram_tensor("band", [128, BAND_W], Hh, kind="ExternalInput")
    out = nc.dram_tensor("out", [1024, 1024], Hh, kind="ExternalOutput")

    AluOp = mybir.AluOpType
    Act = mybir.ActivationFunctionType

    with tile.TileContext(nc) as tc:
        with (
            tc.tile_pool(name="consts", bufs=1) as consts,
            tc.tile_pool(name="xin", bufs=4) as xin,
            tc.tile_pool(name="work", bufs=6) as work,
            tc.tile_pool(name="qpool", bufs=6) as qpool,
            tc.tile_pool(name="keep", bufs=1) as keep,
            tc.tile_pool(name="grayp", bufs=3) as grayp,
            tc.tile_pool(name="tkeep", bufs=8) as tkeep,
            tc.tile_pool(name="vkeep", bufs=1) as vkeep,
            tc.tile_pool(name="maskp", bufs=4) as maskp,
            tc.tile_pool(name="ps1", bufs=1, space="PSUM") as ps1,
            tc.tile_pool(name="ps2", bufs=2, space="PSUM") as ps2,
            tc.tile_pool(name="dram", bufs=1, space="DRAM") as dram,
        ):
            xc = x.ap().rearrange("(i p) (j w) -> p i j w", p=128, w=384)
            # first half-stripe DMA goes before the band const DMA
            xtiles = {}
            xt00 = xin.tile([128, 4, 384], F, tag="xj")
            nc.sync.dma_start(xt00[:], xc[:, 0:4, 0, :])
            xtiles[(0, 0)] = xt00

            band_sb = consts.tile([128, BAND_W], Hh)
            nc.sync.dma_start(band_sb[:], band.ap())
            bias_sq = consts.tile([128, 1], F)
            nc.gpsimd.memset(bias_sq[:], -25.5)
            bias_t1 = consts.tile([128, 1], F)
            nc.gpsimd.memset(bias_t1[:], 1300.5 * 2.0 ** -7)
            gmaxs = consts.tile([1, 9], F)
            gmins = consts.tile([1, 9], F)

            u2all = keep.tile([128, 8, 8, 128], F)       # gray / W0, all pixels
            ta_tiles, tb_tiles = [], []
            v1m, v2m = {}, {}
            for m in range(8):
                t_v1 = vkeep.tile([128, 1024], Hh, name=f"v1_{m}")
                t_v2 = vkeep.tile([128, 1024], Hh, name=f"v2_{m}")
                v1m[m], v2m[m] = t_v1, t_v2

            def emit_p2_unit(m, hc):
                """pass-2 unit: rows block m, cols [512*hc, 512*hc+512)."""
                base = 512 * hc
                ks = H0_KS if hc == 0 else H1_KS
                qa = ps2.tile([128, 512], F, tag="qa")
                qb = ps2.tile([128, 512], F, tag="qb")
                for src_tiles, pt in ((ta_tiles, qa), (tb_tiles, qb)):
                    for n, k in enumerate(ks):
                        jj, c0, ncols = WINDOWS[k]
                        nc.tensor.matmul(
                            pt[:, c0 - base:c0 - base + ncols],
                            src_tiles[jj][:, 128 * m:128 * (m + 1)],
                            band_sb[:, WOFF[k]:WOFF[k] + ncols],
                            start=(n == 0), stop=(n == len(ks) - 1))
                # t1 = ((qa - 1300.5) * 2^-7)^2   (fp16)
                t1 = work.tile([128, 512], Hh, tag="t1")
                nc.scalar.activation(t1[:], qa[:], Act.Square,
                                     bias=bias_t1[:], scale=2.0 ** -7)
                # t2 = qb*2^-14 - t1  = 2601^2 var * 2^-14  (fp16)
                t2 = work.tile([128, 512], Hh, tag="t2")
                nc.vector.scalar_tensor_tensor(
                    t2[:], qb[:], 2.0 ** -14, t1[:],
                    op0=AluOp.mult, op1=AluOp.subtract)
                # s0 = sqrt(t2) = 2601 * s * 2^-7  (fp16)
                s0 = work.tile([128, 512], Hh, tag="s0")
                nc.scalar.activation(s0[:], t2[:], Act.Sqrt, scale=1.0)
                v1 = v1m[m][:, 512 * hc:512 * hc + 512]
                u2slice = u2all[:, m, 4 * hc:4 * hc + 4, :].rearrange(
                    "p a b -> p (a b)")
                nc.vector.scalar_tensor_tensor(
                    v1, qa[:], P0PP, u2slice,
                    op0=AluOp.mult, op1=AluOp.add)
                # v2C = CV2 * s0 * qa  (fp16)
                v2 = v2m[m][:, 512 * hc:512 * hc + 512]
                nc.vector.scalar_tensor_tensor(
                    v2, s0[:], -CV2, qa[:],
                    op0=AluOp.mult, op1=AluOp.mult)

            # ---------------- pass 1: 16 half-stripes ----------------
            for j in range(8):
                tab = ps1.tile([128, 2048], F, tag="AB")
                xa = xtiles.get((j, 0))
                if xa is None:
                    xa = xin.tile([128, 4, 384], F, tag="xj")
                    nc.sync.dma_start(xa[:], xc[:, 0:4, j, :])
                xb = xin.tile([128, 4, 384], F, tag="xj")
                nc.sync.dma_start(xb[:], xc[:, 4:8, j, :])
                s3a = xa[:].rearrange("p i (w c) -> p i w c", c=3)
                s3b = xb[:].rearrange("p i (w c) -> p i w c", c=3)

                u1 = work.tile([128, 8, 128], F, tag="u1")
                u2 = u2all[:, :, j, :]
                gray = grayp.tile([128, 8, 128], Hh, tag="gray")
                g2c = grayp.tile([128, 8, 128], Hh, tag="g2c")
                halves = ((0, s3a), (1, s3b)) if j == 7 else ((2, None),)
                for hh, s3h in halves:
                    if hh == 2:   # full-stripe ops
                        sl = slice(0, 8)
                        src1, src0 = s3a, s3b
                        # u2 positive; gray tile holds -gray (Square is
                        # even, PE/pass-2 absorb the sign in scalars)
                        nc.vector.scalar_tensor_tensor(
                            u1[:, 0:4], s3a[:, :, :, 1], W1 / W0,
                            s3a[:, :, :, 0], op0=AluOp.mult, op1=AluOp.add)
                        nc.vector.scalar_tensor_tensor(
                            u1[:, 4:8], s3b[:, :, :, 1], W1 / W0,
                            s3b[:, :, :, 0], op0=AluOp.mult, op1=AluOp.add)
                        nc.vector.scalar_tensor_tensor(
                            u2[:, 0:4], s3a[:, :, :, 2], W2 / W0, u1[:, 0:4],
                            op0=AluOp.mult, op1=AluOp.add)
                        nc.vector.scalar_tensor_tensor(
                            u2[:, 4:8], s3b[:, :, :, 2], W2 / W0, u1[:, 4:8],
                            op0=AluOp.mult, op1=AluOp.add)
                        # exact f32 max over u2 fires without waiting gray
                        nc.gpsimd.tensor_reduce(
                            gmaxs[0:1, j:j + 1], u2,
                            mybir.AxisListType.XYZWC, AluOp.max)
                        nc.vector.tensor_scalar(gray[:], u2, -W0, None,
                                                op0=AluOp.mult)
                        nc.scalar.activation(g2c[:], gray[:], Act.Square,
                                             bias=bias_sq[:], scale=-51.0)
                        nc.gpsimd.tensor_reduce(
                            gmins[0:1, j:j + 1], gray[:],
                            mybir.AxisListType.XYZWC, AluOp.max)
                    else:         # stripe 7: half-granularity for a short tail
                        sl = slice(4 * hh, 4 * hh + 4)
                        nc.vector.scalar_tensor_tensor(
                            u1[:, sl], s3h[:, :, :, 1], W1 / W0,
                            s3h[:, :, :, 0], op0=AluOp.mult, op1=AluOp.add)
                        nc.vector.scalar_tensor_tensor(
                            u2[:, sl], s3h[:, :, :, 2], W2 / W0, u1[:, sl],
                            op0=AluOp.mult, op1=AluOp.add)
                        with tc.high_priority():
                            nc.gpsimd.tensor_reduce(
                                gmaxs[0:1, 8:9] if hh else gmaxs[0:1, 7:8],
                                u2[:, sl], mybir.AxisListType.XYZWC,
                                AluOp.max)
                        nc.vector.tensor_scalar(gray[:, sl], u2[:, sl], -W0,
                                                None, op0=AluOp.mult)
                        nc.scalar.activation(g2c[:, sl], gray[:, sl],
                                             Act.Square, bias=bias_sq[:],
                                             scale=-51.0)
                        with tc.high_priority():
                            nc.gpsimd.tensor_reduce(
                                gmins[0:1, 8:9] if hh else gmins[0:1, 7:8],
                                gray[:, sl], mybir.AxisListType.XYZWC,
                                AluOp.max)

                # H-pass banded matmuls
                for src, off in ((gray, 0), (g2c, 1024)):
                    for k, (i, c0, ncols) in enumerate(WINDOWS):
                        nc.tensor.matmul(
                            tab[:, off + c0:off + c0 + ncols],
                            src[:, i, :],
                            band_sb[:, WOFF[k]:WOFF[k] + ncols],
                            start=(k in (0, 4)), stop=(k in (5, 9)))
                # merged ta|tb fp16 copy
                tabh = tkeep.tile([128, 2048], Hh, tag="tab")
                nc.scalar.copy(tabh[:], tab[:])
                ta_tiles.append(tabh[:, 0:1024])
                tb_tiles.append(tabh[:, 1024:2048])


            # ------------- global r (AllGather of (gmax, -gmin)) ------
            # entire final chain on POOL: its queue is empty here, while
            # DVE's exec queue is flooded with pass-2 unit work
            with tc.high_priority():
                # gmaxs slots are gray units; gmins slots hold max(-u2)=-gmin
                gmaxu = consts.tile([1, 1], F)
                nc.gpsimd.tensor_reduce(gmaxu[:], gmaxs[:],
                                        mybir.AxisListType.XYZWC, AluOp.max)
                gminu = consts.tile([1, 1], F)
                nc.gpsimd.tensor_reduce(gminu[:], gmins[:],
                                        mybir.AxisListType.XYZWC, AluOp.max)
                pairg = consts.tile([1, 2], F)
                nc.gpsimd.tensor_scalar(pairg[:, 0:1], gmaxu[:], W0, None,
                                        op0=AluOp.mult)
                nc.gpsimd.tensor_scalar(pairg[:, 1:2], gminu[:], 1.0, None,
                                        op0=AluOp.mult)
                mm_in = dram.tile([1, 2], F)
                mm_sh = dram.tile([1, 16], F, addr_space="Shared")
                nc.sync.dma_start(mm_in[:], pairg[:])
                nc.gpsimd.collective_compute(
                    "AllGather", AluOp.bypass,
                    replica_groups=[list(range(N_CORES))],
                    ins=[mm_in.opt()], outs=[mm_sh.opt()])

            # ---- remaining pass-2 units (before the r-dependent chain,
            # so DVE/ACT streams do not block on the collective) ----
            for m in range(0, 8):
                emit_p2_unit(m, 0)
            for m in range(8):
                emit_p2_unit(m, 1)

            # ---- r-dependent chain ----
            with tc.high_priority():
                mm_b = consts.tile([128, 16], F)
                nc.sync.dma_start(mm_b[:], mm_sh[:].to_broadcast((128, 16)))
                glob = consts.tile([128, 2], F)
                nc.vector.tensor_reduce(
                    glob[:], mm_b[:].rearrange("p (s c) -> p c s", c=2),
                    mybir.AxisListType.X, AluOp.max)
                rs1 = consts.tile([128, 1], F)
                nc.vector.tensor_tensor(rs1[:], glob[:, 0:1], glob[:, 1:2],
                                        op=AluOp.add)
                rsum2 = consts.tile([128, 1], F)
                nc.vector.tensor_scalar(rsum2[:], rs1[:], RS_SCALE, None,
                                        op0=AluOp.mult)

            # masks: w = v1*rsum2 (ACT, per-partition scale), then
            # mask = w > v2C (DVE fp16 TT 2x)
            out_r = out.ap().rearrange("(m p) w -> m p w", p=128)
            for m in range(8):
                w = maskp.tile([128, 1024], Hh, tag="w")
                nc.scalar.activation(w[:], v1m[m][:], Act.Copy,
                                     scale=rsum2[:])
                mask = maskp.tile([128, 1024], Hh, tag="mask")
                nc.vector.tensor_tensor(mask[:], w[:], v2m[m][:],
                                        op=AluOp.is_gt)
                nc.sync.dma_start(out_r[m], mask[:])

    _split_multi_waits(nc)
    return nc


_CACHE = {}


def _get_nc():
    if "nc" not in _CACHE:
        _CACHE["nc"] = _build_nc()
        _CACHE["band"] = _build_band_blocks()
    return _CACHE["nc"], _CACHE["band"]


def kernel(inputs: np.ndarray) -> np.ndarray:
    nc, band = _get_nc()
    x = np.asarray(inputs, dtype=np.float32)
    in_maps = [
        {"x": np.ascontiguousarray(x[c].reshape(1024, 3072)), "band": band}
        for c in range(N_CORES)
    ]
    res = run_bass_kernel_spmd(nc, in_maps, list(range(N_CORES)))
    masks = [res.results[c]["out"] for c in range(N_CORES)]
    return np.stack(masks)[..., None].astype(np.float32)


# revision 5
# speedup vs baseline: 1.0022x; 1.0010x over previous
"""Sauvola binarization kernel v2 for 8 Trainium2 NeuronCores (data-parallel).

Structure (per core, one 1024x1024x3 image):
  pass 1 over 16 half-stripes (w-stripe j, row-half h): DMA [128,4,384],
    u1/u2 grayscale (DVE f32, some u1 on POOL), gray16 cast (DVE),
    g2c=(51g-25.5)^2 (ACT), exact f32 min/max folds (POOL),
    banded H-boxsum matmuls (PE) -> merged ta|tb fp16 copy (ACT).
  global r: DVE final reduces -> partition_all_reduce (POOL) -> AllGather
    (15us, vs 28us AllReduce) -> broadcast-read -> slot max-reduce.
    1/(2r) is folded into the mask compare: mask = (v1 * rsum2) > CV2*s0*qa.
  pass 2 in (m-block, col-half) units of [128,512] PSUM: W-boxsum matmuls
    (PE), t1/sqrt (ACT), t2/v1/v2 (DVE stt), masks split DVE/POOL.
  Left-half units only need stripes 0..4, so they overlap pass 1.
"""
import numpy as np

import concourse.bass as bass
import concourse.mybir as mybir
import concourse.tile as tile
from concourse.bass_utils import run_bass_kernel_spmd

N_CORES = 8
F = mybir.dt.float32
Hh = mybir.dt.float16
W0, W1, W2 = 0.2989, 0.5870, 0.1140
KS = 0.2
HALF = 25
WINDOWS = [(0, 0, 153), (1, 103, 178), (2, 231, 178), (3, 359, 153), (3, 512, 25),
           (4, 487, 25), (4, 512, 153), (5, 615, 178), (6, 743, 178), (7, 871, 153)]
WOFF = [0]
for _i, _c, _n in WINDOWS[:-1]:
    WOFF.append(WOFF[-1] + _n)
BAND_W = WOFF[-1] + WINDOWS[-1][2]   # 1374
# pass-2 col halves: pieces whose output range is in [0,512) vs [512,1024)
H0_KS = [0, 1, 2, 3, 5]
H1_KS = [4, 6, 7, 8, 9]
P0PP = (1.0 - KS) / (2601.0 * W0)
# mask:  v1 * ((maxu2-minu2)*W0/K2) > CV2*s0*qa,  s0 = 2601*s*2^-7
CV2 = 2.0 ** -4
K2 = 2.0 * KS * (2 ** 7) / (W0 * 2601.0 ** 2 * CV2)
RS_SCALE = 1.0 / K2   # pair is in gray units
U1_POOL_STRIPES = ()                 # u1 stays on DVE
MASK_POOL_UNITS = 6                  # of 16 mask units, this many on POOL


def _split_multi_waits(nc):
    """walrus here allows one sync wait per instruction; split extras to NOPs."""
    for func in nc.m.functions:
        for bb in func.blocks:
            insts = bb.instructions
            i = 0
            while i < len(insts):
                inst = insts[i]
                si = inst.sync_info
                if si is None or len(si.on_wait) <= 1:
                    i += 1
                    continue
                waits = list(si.on_wait)
                nops = []
                for w in waits[:-1]:
                    nop = mybir.InstNoOp(
                        name=nc.get_next_instruction_name(),
                        sync_info=mybir.SyncInfo(on_wait=[w], on_update=[]),
                        bass_nofuse=True,
                        engine=inst.engine,
                    )
                    nops.append(nop)
                inst.sync_info = mybir.SyncInfo(
                    on_wait=[waits[-1]], on_update=list(si.on_update)
                )
                for k, nop in enumerate(nops):
                    insts.insert(i + k, nop)
                    nc.register_instruction(nop, overwrite=True)
                i += len(nops) + 1


def _build_band_blocks():
    B = np.zeros((1024, 1024), dtype=np.float32)
    idx = np.arange(1024)
    for d in range(-HALF, HALF + 1):
        t = idx + d
        t = np.where(t < 0, -t, t)
        t = np.where(t > 1023, 2046 - t, t)
        np.add.at(B, (idx, t), 1.0)
    blocks = np.zeros((128, BAND_W), dtype=np.float16)
    for k, (i, c0, ncols) in enumerate(WINDOWS):
        blocks[:, WOFF[k]:WOFF[k] + ncols] = B[c0:c0 + ncols,
                                               128 * i:128 * (i + 1)].T[:, :]
    return blocks


def _build_nc():
    nc = bass.Bass("TRN2", target_bir_lowering=False, debug=False,
                   num_devices=N_CORES)
    x = nc.dram_tensor("x", [1024, 3072], F, kind="ExternalInput")
    band = nc.dram_tensor("band", [128, BAND_W], Hh, kind="ExternalInput")
    out = nc.dram_tensor("out", [1024, 1024], Hh, kind="ExternalOutput")

    AluOp = mybir.AluOpType
    Act = mybir.ActivationFunctionType

    with tile.TileContext(nc) as tc:
        with (
            tc.tile_pool(name="consts", bufs=1) as consts,
            tc.tile_pool(name="xin", bufs=4) as xin,
            tc.tile_pool(name="work", bufs=6) as work,
            tc.tile_pool(name="qpool", bufs=6) as qpool,
            tc.tile_pool(name="keep", bufs=1) as keep,
            tc.tile_pool(name="grayp", bufs=3) as grayp,
            tc.tile_pool(name="tkeep", bufs=8) as tkeep,
            tc.tile_pool(name="vkeep", bufs=1) as vkeep,
            tc.tile_pool(name="maskp", bufs=4) as maskp,
            tc.tile_pool(name="ps1", bufs=1, space="PSUM") as ps1,
            tc.tile_pool(name="ps2", bufs=2, space="PSUM") as ps2,
            tc.tile_pool(name="dram", bufs=1, space="DRAM") as dram,
        ):
            xc = x.ap().rearrange("(i p) (j w) -> p i j w", p=128, w=384)
            # first half-stripe DMA goes before the band const DMA
            xtiles = {}
            xt00 = xin.tile([128, 4, 384], F, tag="xj")
            nc.sync.dma_start(xt00[:], xc[:, 0:4, 0, :])
            xtiles[(0, 0)] = xt00

            band_sb = consts.tile([128, BAND_W], Hh)
            nc.sync.dma_start(band_sb[:], band.ap())
            bias_sq = consts.tile([128, 1], F)
            nc.gpsimd.memset(bias_sq[:], -25.5)
            bias_t1 = consts.tile([128, 1], F)
            nc.gpsimd.memset(bias_t1[:], 1300.5 * 2.0 ** -7)
            gmaxs = consts.tile([1, 11], F)
            gmins = consts.tile([1, 11], F)

            u2all = keep.tile([128, 8, 8, 128], F)       # gray / W0, all pixels
            ta_tiles, tb_tiles = [], []
            v1m, v2m = {}, {}
            for m in range(8):
                t_v1 = vkeep.tile([128, 1024], Hh, name=f"v1_{m}")
                t_v2 = vkeep.tile([128, 1024], Hh, name=f"v2_{m}")
                v1m[m], v2m[m] = t_v1, t_v2

            def emit_p2_unit(m, hc):
                """pass-2 unit: rows block m, cols [512*hc, 512*hc+512)."""
                base = 512 * hc
                ks = H0_KS if hc == 0 else H1_KS
                qa = ps2.tile([128, 512], F, tag="qa")
                qb = ps2.tile([128, 512], F, tag="qb")
                for src_tiles, pt in ((ta_tiles, qa), (tb_tiles, qb)):
                    for n, k in enumerate(ks):
                        jj, c0, ncols = WINDOWS[k]
                        nc.tensor.matmul(
                            pt[:, c0 - base:c0 - base + ncols],
                            src_tiles[jj][:, 128 * m:128 * (m + 1)],
                            band_sb[:, WOFF[k]:WOFF[k] + ncols],
                            start=(n == 0), stop=(n == len(ks) - 1))
                # t1 = ((qa - 1300.5) * 2^-7)^2   (fp16)
                t1 = work.tile([128, 512], Hh, tag="t1")
                nc.scalar.activation(t1[:], qa[:], Act.Square,
                                     bias=bias_t1[:], scale=2.0 ** -7)
                # t2 = qb*2^-14 - t1  = 2601^2 var * 2^-14  (fp16)
                t2 = work.tile([128, 512], Hh, tag="t2")
                nc.vector.scalar_tensor_tensor(
                    t2[:], qb[:], 2.0 ** -14, t1[:],
                    op0=AluOp.mult, op1=AluOp.subtract)
                # s0 = sqrt(t2) = 2601 * s * 2^-7  (fp16)
                s0 = work.tile([128, 512], Hh, tag="s0")
                nc.scalar.activation(s0[:], t2[:], Act.Sqrt, scale=1.0)
                v1 = v1m[m][:, 512 * hc:512 * hc + 512]
                u2slice = u2all[:, m, 4 * hc:4 * hc + 4, :].rearrange(
                    "p a b -> p (a b)")
                nc.vector.scalar_tensor_tensor(
                    v1, qa[:], P0PP, u2slice,
                    op0=AluOp.mult, op1=AluOp.add)
                # v2C = CV2 * s0 * qa  (fp16)
                v2 = v2m[m][:, 512 * hc:512 * hc + 512]
                nc.vector.scalar_tensor_tensor(
                    v2, s0[:], -CV2, qa[:],
                    op0=AluOp.mult, op1=AluOp.mult)

            # ---------------- pass 1: 16 half-stripes ----------------
            for j in range(8):
                tab = ps1.tile([128, 2048], F, tag="AB")
                xa = xtiles.get((j, 0))
                if xa is None:
                    xa = xin.tile([128, 4, 384], F, tag="xj")
                    nc.sync.dma_start(xa[:], xc[:, 0:4, j, :])
                xb = xin.tile([128, 4, 384], F, tag="xj")
                nc.sync.dma_start(xb[:], xc[:, 4:8, j, :])
                s3a = xa[:].rearrange("p i (w c) -> p i w c", c=3)
                s3b = xb[:].rearrange("p i (w c) -> p i w c", c=3)

                u1 = work.tile([128, 8, 128], F, tag="u1")
                u2 = u2all[:, :, j, :]
                gray = grayp.tile([128, 8, 128], Hh, tag="gray")
                g2c = grayp.tile([128, 8, 128], Hh, tag="g2c")
                halves = ((0, s3a), (1, s3b)) if j == 7 else ((2, None),)
                for hh, s3h in halves:
                    if hh == 2:   # full-stripe ops
                        sl = slice(0, 8)
                        src1, src0 = s3a, s3b
                        # u2 positive; gray tile holds -gray (Square is
                        # even, PE/pass-2 absorb the sign in scalars)
                        nc.vector.scalar_tensor_tensor(
                            u1[:, 0:4], s3a[:, :, :, 1], W1 / W0,
                            s3a[:, :, :, 0], op0=AluOp.mult, op1=AluOp.add)
                        nc.vector.scalar_tensor_tensor(
                            u1[:, 4:8], s3b[:, :, :, 1], W1 / W0,
                            s3b[:, :, :, 0], op0=AluOp.mult, op1=AluOp.add)
                        nc.vector.scalar_tensor_tensor(
                            u2[:, 0:4], s3a[:, :, :, 2], W2 / W0, u1[:, 0:4],
                            op0=AluOp.mult, op1=AluOp.add)
                        nc.vector.scalar_tensor_tensor(
                            u2[:, 4:8], s3b[:, :, :, 2], W2 / W0, u1[:, 4:8],
                            op0=AluOp.mult, op1=AluOp.add)
                        # exact f32 max over u2 fires without waiting gray
                        nc.gpsimd.tensor_reduce(
                            gmaxs[0:1, j:j + 1], u2,
                            mybir.AxisListType.XYZWC, AluOp.max)
                        nc.vector.tensor_scalar(gray[:], u2, -W0, None,
                                                op0=AluOp.mult)
                        nc.scalar.activation(g2c[:], gray[:], Act.Square,
                                             bias=bias_sq[:], scale=-51.0)
                        nc.gpsimd.tensor_reduce(
                            gmins[0:1, j:j + 1], gray[:],
                            mybir.AxisListType.XYZWC, AluOp.max)
                    else:         # stripe 7: quarter tail for a short chain
                        quarters = ((7, slice(4 * hh, 4 * hh + 2)),
                                    (9, slice(4 * hh + 2, 4 * hh + 4))) \
                            if hh == 0 else \
                                   ((8, slice(4 * hh, 4 * hh + 2)),
                                    (10, slice(4 * hh + 2, 4 * hh + 4)))
                        for slot, sq in quarters:
                            nc.vector.scalar_tensor_tensor(
                                u1[:, sq], s3h[:, :, :, 1][:, sq.start - 4 * hh:sq.stop - 4 * hh],
                                W1 / W0,
                                s3h[:, :, :, 0][:, sq.start - 4 * hh:sq.stop - 4 * hh],
                                op0=AluOp.mult, op1=AluOp.add)
                            nc.vector.scalar_tensor_tensor(
                                u2[:, sq],
                                s3h[:, :, :, 2][:, sq.start - 4 * hh:sq.stop - 4 * hh],
                                W2 / W0, u1[:, sq],
                                op0=AluOp.mult, op1=AluOp.add)
                            with tc.high_priority():
                                nc.gpsimd.tensor_reduce(
                                    gmaxs[0:1, slot:slot + 1], u2[:, sq],
                                    mybir.AxisListType.XYZWC, AluOp.max)
                            nc.vector.tensor_scalar(gray[:, sq], u2[:, sq],
                                                    -W0, None,
                                                    op0=AluOp.mult)
                            nc.scalar.activation(g2c[:, sq], gray[:, sq],
                                                 Act.Square, bias=bias_sq[:],
                                                 scale=-51.0)
                            with tc.high_priority():
                                nc.gpsimd.tensor_reduce(
                                    gmins[0:1, slot:slot + 1], gray[:, sq],
                                    mybir.AxisListType.XYZWC, AluOp.max)

                # H-pass banded matmuls
                for src, off in ((gray, 0), (g2c, 1024)):
                    for k, (i, c0, ncols) in enumerate(WINDOWS):
                        nc.tensor.matmul(
                            tab[:, off + c0:off + c0 + ncols],
                            src[:, i, :],
                            band_sb[:, WOFF[k]:WOFF[k] + ncols],
                            start=(k in (0, 4)), stop=(k in (5, 9)))
                # merged ta|tb fp16 copy
                tabh = tkeep.tile([128, 2048], Hh, tag="tab")
                nc.scalar.copy(tabh[:], tab[:])
                ta_tiles.append(tabh[:, 0:1024])
                tb_tiles.append(tabh[:, 1024:2048])


            # ------------- global r (AllGather of (gmax, -gmin)) ------
            # entire final chain on POOL: its queue is empty here, while
            # DVE's exec queue is flooded with pass-2 unit work
            with tc.high_priority():
                # gmaxs slots are gray units; gmins slots hold max(-u2)=-gmin
                gmaxu = consts.tile([1, 1], F)
                nc.gpsimd.tensor_reduce(gmaxu[:], gmaxs[:],
                                        mybir.AxisListType.XYZWC, AluOp.max)
                gminu = consts.tile([1, 1], F)
                nc.gpsimd.tensor_reduce(gminu[:], gmins[:],
                                        mybir.AxisListType.XYZWC, AluOp.max)
                pairg = consts.tile([1, 2], F)
                nc.gpsimd.tensor_scalar(pairg[:, 0:1], gmaxu[:], W0, None,
                                        op0=AluOp.mult)
                nc.gpsimd.tensor_scalar(pairg[:, 1:2], gminu[:], 1.0, None,
                                        op0=AluOp.mult)
                mm_in = dram.tile([1, 2], F)
                mm_sh = dram.tile([1, 16], F, addr_space="Shared")
                nc.sync.dma_start(mm_in[:], pairg[:])
                nc.gpsimd.collective_compute(
                    "AllGather", AluOp.bypass,
                    replica_groups=[list(range(N_CORES))],
                    ins=[mm_in.opt()], outs=[mm_sh.opt()])

            # ---- remaining pass-2 units (before the r-dependent chain,
            # so DVE/ACT streams do not block on the collective) ----
            for m in range(0, 8):
                emit_p2_unit(m, 0)
            for m in range(8):
                emit_p2_unit(m, 1)

            # ---- r-dependent chain ----
            with tc.high_priority():
                mm_b = consts.tile([128, 16], F)
                nc.sync.dma_start(mm_b[:], mm_sh[:].to_broadcast((128, 16)))
                glob = consts.tile([128, 2], F)
                nc.vector.tensor_reduce(
                    glob[:], mm_b[:].rearrange("p (s c) -> p c s", c=2),
                    mybir.AxisListType.X, AluOp.max)
                rs1 = consts.tile([128, 1], F)
                nc.vector.tensor_tensor(rs1[:], glob[:, 0:1], glob[:, 1:2],
                                        op=AluOp.add)
                rsum2 = consts.tile([128, 1], F)
                nc.vector.tensor_scalar(rsum2[:], rs1[:], RS_SCALE, None,
                                        op0=AluOp.mult)

            # masks: w = v1*rsum2 (ACT, per-partition scale), then
            # mask = w > v2C (DVE fp16 TT 2x)
            out_r = out.ap().rearrange("(m p) w -> m p w", p=128)
            for m in range(8):
                w = maskp.tile([128, 1024], Hh, tag="w")
                nc.scalar.activation(w[:], v1m[m][:], Act.Copy,
                                     scale=rsum2[:])
                mask = maskp.tile([128, 1024], Hh, tag="mask")
                nc.vector.tensor_tensor(mask[:], w[:], v2m[m][:],
                                        op=AluOp.is_gt)
                nc.sync.dma_start(out_r[m], mask[:])

    _split_multi_waits(nc)
    return nc


_CACHE = {}


def _get_nc():
    if "nc" not in _CACHE:
        _CACHE["nc"] = _build_nc()
        _CACHE["band"] = _build_band_blocks()
    return _CACHE["nc"], _CACHE["band"]


def kernel(inputs: np.ndarray) -> np.ndarray:
    nc, band = _get_nc()
    x = np.asarray(inputs, dtype=np.float32)
    in_maps = [
        {"x": np.ascontiguousarray(x[c].reshape(1024, 3072)), "band": band}
        for c in range(N_CORES)
    ]
    res = run_bass_kernel_spmd(nc, in_maps, list(range(N_CORES)))
    masks = [res.results[c]["out"] for c in range(N_CORES)]
    return np.stack(masks)[..., None].astype(np.float32)
